# revision 1
# baseline (speedup 1.0000x reference)
"""Trainium2 Bass kernel for nn_Deep_Mem_AbsRelate_SparseCOO (scatter_memory).

The 16-dim COO coords are all in {0,1}, so every row linearizes to a unique
16-bit key: two rows collide under the reference's mixed-radix strides iff
they are bit-identical, i.e. iff they share the binary key  k = sum_d c_d 2^d.
The task is therefore a 65536-bin weighted histogram of the stored rows
followed by a per-query lookup.

Plan (8 NeuronCores, data-parallel):
  Launch A: each core histograms its 1/8 of the stores into a [128 hi, 512 lo]
            PSUM tile via one-hot matmuls (bin = hi*512 + lo, hi = key>>9).
            VectorE builds keys + 512-wide lo one-hots; ScalarE builds the
            128-wide hi one-hots via a 2-pass relu(1-|iota-key|) trick;
            TensorE accumulates onehot_hi^T @ onehot_lo.
  Host:     sums the 8 partial histograms (256 KB each).
  Launch B: each core answers its 1/8 of the queries: G = onehot_hi^T @ HIST
            (TensorE, via a PE transpose of the hi one-hot) gives each query's
            512-wide hist row; a fused multiply+reduce (tensor_tensor_reduce)
            against the lo one-hot selects the answer.

NOTE: the walrus build here accepts at most ONE sync-wait per instruction and
does not populate extended-ISA instruction bytes -- _split_waits() and
lower_extended_insts() below patch both after Tile scheduling.
"""

import numpy as np

import concourse.bass as bass
import concourse.mybir as mybir
from concourse.tile import TileContext
from concourse.bass_utils import run_bass_kernel_spmd
from concourse.library_overlay import lower_extended_insts

P = 128          # SBUF partitions
C = 16           # items per partition per chunk
CHUNK = P * C    # 2048 items per chunk
W = 32           # int32 lanes per item (16 int64 coords)
NCORES = 8
F16 = mybir.dt.float16
F32 = mybir.dt.float32
I32 = mybir.dt.int32
AX = mybir.AxisListType.X
OP = mybir.AluOpType
AF = mybir.ActivationFunctionType


def _split_waits(nc):
    """walrus in this toolchain accepts at most ONE sync-wait per instruction.
    Tile's sem-assignment attaches several; move the excess onto standalone
    InstEventSemaphore instructions just before, on the same engine."""
    for f in nc.m.functions:
        for blk in f.blocks:
            insts = list(blk.instructions)
            out = []
            changed = False
            for inst in insts:
                si = inst.sync_info
                if si is not None and si.on_wait and len(si.on_wait) > 1:
                    waits = list(si.on_wait)
                    for w in waits[:-1]:
                        ev = mybir.InstEventSemaphore(
                            name=f"WSPLIT-{nc.next_id()}", ins=[], outs=[]
                        )
                        ev.engine = inst.engine
                        ev.sync_info = mybir.SyncInfo(on_wait=[w], on_update=[])
                        out.append(ev)
                    inst.sync_info = mybir.SyncInfo(
                        on_wait=waits[-1:], on_update=list(si.on_update)
                    )
                    changed = True
                out.append(inst)
            if changed:
                blk.instructions = out


def _wmul_np():
    # int32 lane weights: lane 2d = weight of coord d, lane 2d+1 = 0 (the
    # int64 high words, always 0 for 0/1 coords). lo = bits 0..8, hi = 9..15.
    w = np.zeros(W, np.int32)
    for d in range(16):
        w[2 * d] = (1 << d) if d < 9 else (1 << (d - 9))
    return np.tile(w, (P, C)).copy()


def _iota_np(n):
    return np.tile(np.arange(n, dtype=np.float16), (P, 1)).copy()


def _keys(nc, cc, WM, work, keysp, negate_hi):
    """int32 coords chunk [P, C*W] -> fp32 lo ([P,C], 9-bit) and hi ([P,C],
    7-bit, optionally negated) key parts."""
    prod = work.tile([P, C * W], I32, tag="prod")
    nc.vector.tensor_tensor(out=prod[:], in0=cc[:], in1=WM[:], op=OP.mult)
    pv = prod[:].rearrange("p (c w) -> p c w", w=W)
    lo32 = keysp.tile([P, C], F32, tag="lo32")
    hi32 = keysp.tile([P, C], F32, tag="hi32")
    # lanes 0..17 carry coords 0..8 (lo), lanes 18..31 carry coords 9..15 (hi)
    nc.vector.tensor_reduce(out=lo32[:], in_=pv[:, :, 0:18], axis=AX, op=OP.add)
    nc.vector.tensor_reduce(
        out=hi32[:], in_=pv[:, :, 18:32], axis=AX, op=OP.add, negate=negate_hi
    )
    return lo32, hi32


def build_store(nblk):
    nc = bass.Bass("TRN2")
    coords = nc.dram_tensor("coords", [nblk, P, C * W], I32, kind="ExternalInput")
    vals = nc.dram_tensor("vals", [nblk, P, C], F32, kind="ExternalInput")
    wmul = nc.dram_tensor("wmul", [P, C * W], I32, kind="ExternalInput")
    iota_lo = nc.dram_tensor("iota_lo", [P, 512], F16, kind="ExternalInput")
    iota_hi = nc.dram_tensor("iota_hi", [P, 128], F16, kind="ExternalInput")
    hist = nc.dram_tensor("hist", [P, 512], F32, kind="ExternalOutput")

    with TileContext(nc) as tc:
        with (
            tc.tile_pool(name="const", bufs=1) as constp,
            tc.tile_pool(name="cin", bufs=3) as cin,
            tc.tile_pool(name="vin", bufs=3) as vin,
            tc.tile_pool(name="work", bufs=2) as work,
            tc.tile_pool(name="keys", bufs=3) as keysp,
            tc.tile_pool(name="oh", bufs=4) as ohp,
            tc.tile_pool(name="ohh", bufs=4) as ohhp,
            tc.tile_pool(name="psum", bufs=1, space="PSUM") as psump,
            tc.tile_pool(name="outp", bufs=1) as outp,
        ):
            WM = constp.tile([P, C * W], I32)
            nc.sync.dma_start(out=WM[:], in_=wmul[:, :])
            IL = constp.tile([P, 512], F16)
            nc.sync.dma_start(out=IL[:], in_=iota_lo[:, :])
            IH = constp.tile([P, 128], F16)
            nc.sync.dma_start(out=IH[:], in_=iota_hi[:, :])

            hp = psump.tile([P, 512], F32)
            total = nblk * C
            jg = 0
            for b in range(nblk):
                cc = cin.tile([P, C * W], I32)
                nc.sync.dma_start(out=cc[:], in_=coords[b])
                vv = vin.tile([P, C], F32)
                nc.sync.dma_start(out=vv[:], in_=vals[b])
                lof, neghi = _keys(nc, cc, WM, work, keysp, negate_hi=True)
                for j in range(C):
                    loh = ohp.tile([P, 512], F16, tag="loh")
                    nc.vector.tensor_scalar(
                        out=loh[:], in0=IL[:],
                        scalar1=lof[:, j : j + 1], scalar2=vv[:, j : j + 1],
                        op0=OP.is_equal, op1=OP.mult,
                    )
                    # hi one-hot on ScalarE: relu(1 - |iota - hi|)
                    t1 = ohhp.tile([P, 128], F16, tag="t1")
                    nc.scalar.activation(
                        out=t1[:], in_=IH[:], func=AF.Abs,
                        bias=neghi[:, j : j + 1], scale=1.0,
                    )
                    hih = ohhp.tile([P, 128], F16, tag="hih")
                    nc.scalar.activation(
                        out=hih[:], in_=t1[:], func=AF.Relu, bias=1.0, scale=-1.0,
                    )
                    nc.tensor.matmul(
                        out=hp[:], lhsT=hih[:], rhs=loh[:],
                        start=(jg == 0), stop=(jg == total - 1),
                    )
                    jg += 1
            hs = outp.tile([P, 512], F32)
            nc.vector.tensor_copy(hs[:], hp[:])
            nc.scalar.dma_start(out=hist[:, :], in_=hs[:])
    _split_waits(nc)
    lower_extended_insts(nc)
    return nc


def build_query(nblk):
    nc = bass.Bass("TRN2")
    queries = nc.dram_tensor("queries", [nblk, P, C * W], I32, kind="ExternalInput")
    hist16 = nc.dram_tensor("hist16", [P, 512], F16, kind="ExternalInput")
    wmul = nc.dram_tensor("wmul", [P, C * W], I32, kind="ExternalInput")
    iota_lo = nc.dram_tensor("iota_lo", [P, 512], F16, kind="ExternalInput")
    iota_hi32 = nc.dram_tensor("iota_hi32", [P, 128], F32, kind="ExternalInput")
    ident = nc.dram_tensor("ident", [P, 128], F32, kind="ExternalInput")
    ans = nc.dram_tensor("ans", [nblk, P, C], F32, kind="ExternalOutput")

    with TileContext(nc) as tc:
        with (
            tc.tile_pool(name="const", bufs=1) as constp,
            tc.tile_pool(name="cin", bufs=3) as cin,
            tc.tile_pool(name="work", bufs=2) as work,
            tc.tile_pool(name="keys", bufs=3) as keysp,
            tc.tile_pool(name="oh", bufs=4) as ohp,
            tc.tile_pool(name="tp", bufs=2, space="PSUM") as tpp,
            tc.tile_pool(name="gp", bufs=2, space="PSUM") as gpp,
            tc.tile_pool(name="hT", bufs=3) as hTp,
            tc.tile_pool(name="scr", bufs=2) as scrp,
            tc.tile_pool(name="ansp", bufs=3) as ansp,
        ):
            WM = constp.tile([P, C * W], I32)
            nc.sync.dma_start(out=WM[:], in_=wmul[:, :])
            IL = constp.tile([P, 512], F16)
            nc.sync.dma_start(out=IL[:], in_=iota_lo[:, :])
            IH32 = constp.tile([P, 128], F32)
            nc.sync.dma_start(out=IH32[:], in_=iota_hi32[:, :])
            IDN = constp.tile([P, 128], F32)
            nc.sync.dma_start(out=IDN[:], in_=ident[:, :])
            HI16 = constp.tile([P, 512], F16)
            nc.sync.dma_start(out=HI16[:], in_=hist16[:, :])

            for b in range(nblk):
                cc = cin.tile([P, C * W], I32)
                nc.sync.dma_start(out=cc[:], in_=queries[b])
                lof, hif = _keys(nc, cc, WM, work, keysp, negate_hi=False)
                ac = ansp.tile([P, C], F32)
                for j in range(C):
                    hih = ohp.tile([P, 128], F32, tag="hih")
                    nc.vector.tensor_scalar(
                        out=hih[:], in0=IH32[:],
                        scalar1=hif[:, j : j + 1], scalar2=None, op0=OP.is_equal,
                    )
                    tps = tpp.tile([P, 128], F32)
                    nc.tensor.transpose(out=tps[:], in_=hih[:], identity=IDN[:])
                    hihT = hTp.tile([P, 128], F16)
                    nc.scalar.copy(hihT[:], tps[:])
                    g = gpp.tile([P, 512], F32)
                    nc.tensor.matmul(out=g[:], lhsT=hihT[:], rhs=HI16[:], start=True, stop=True)
                    loh = ohp.tile([P, 512], F16, tag="loh")
                    nc.vector.tensor_scalar(
                        out=loh[:], in0=IL[:],
                        scalar1=lof[:, j : j + 1], scalar2=None, op0=OP.is_equal,
                    )
                    gs = scrp.tile([P, 512], F16, tag="gs")
                    nc.scalar.copy(gs[:], g[:])
                    scr = scrp.tile([P, 512], F16, tag="scr")
                    nc.vector.tensor_tensor(out=scr[:], in0=gs[:], in1=loh[:], op=OP.mult)
                    nc.vector.tensor_reduce(
                        out=ac[:, j : j + 1], in_=scr[:], axis=AX, op=OP.add
                    )
                nc.scalar.dma_start(out=ans[b], in_=ac[:])
    _split_waits(nc)
    lower_extended_insts(nc)
    return nc


_CACHE = {}


def _get(builder, nblk):
    key = (builder.__name__, nblk)
    if key not in _CACHE:
        _CACHE[key] = builder(nblk)
    return _CACHE[key]


def kernel(stored_coords: np.ndarray, queries: np.ndarray, store_vals: np.ndarray) -> np.ndarray:
    n = stored_coords.shape[0]
    assert queries.shape[0] == n
    percore = -(-n // NCORES)            # ceil
    nblk = -(-percore // CHUNK)          # ceil
    padded = nblk * CHUNK

    wm = _wmul_np()
    il = _iota_np(512)
    ih = _iota_np(128)
    ih32 = np.tile(np.arange(128, dtype=np.float32), (P, 1)).copy()
    idn = np.eye(128, dtype=np.float32)

    sc = np.ascontiguousarray(stored_coords.astype(np.int64, copy=False)).view(np.int32)
    qc = np.ascontiguousarray(queries.astype(np.int64, copy=False)).view(np.int32)
    sv = store_vals.astype(np.float32, copy=False)

    in_a, in_b = [], []
    for c in range(NCORES):
        lo_i, hi_i = c * percore, min((c + 1) * percore, n)
        m = hi_i - lo_i
        cpad = np.zeros((padded, W), np.int32)
        cpad[:m] = sc[lo_i:hi_i]
        vpad = np.zeros((padded,), np.float32)
        vpad[:m] = sv[lo_i:hi_i]
        qpad = np.zeros((padded, W), np.int32)
        qpad[:m] = qc[lo_i:hi_i]
        in_a.append({
            "coords": cpad.reshape(nblk, P, C * W),
            "vals": vpad.reshape(nblk, P, C),
            "wmul": wm, "iota_lo": il, "iota_hi": ih,
        })
        in_b.append({
            "queries": qpad.reshape(nblk, P, C * W),
            "wmul": wm, "iota_lo": il, "iota_hi32": ih32, "ident": idn,
        })

    nc_a = _get(build_store, nblk)
    print("kernel: store launch...", flush=True)
    res_a = run_bass_kernel_spmd(nc_a, in_a, core_ids=list(range(NCORES)))
    hist = np.zeros((P, 512), np.float32)
    for c in range(NCORES):
        hist += res_a.results[c]["hist"]
    h16 = hist.astype(np.float16)
    for mm_ in in_b:
        mm_["hist16"] = h16

    nc_b = _get(build_query, nblk)
    print("kernel: query launch...", flush=True)
    res_b = run_bass_kernel_spmd(nc_b, in_b, core_ids=list(range(NCORES)))

    out = np.empty((n,), np.float32)
    for c in range(NCORES):
        lo_i, hi_i = c * percore, min((c + 1) * percore, n)
        a = res_b.results[c]["ans"].reshape(padded)
        out[lo_i:hi_i] = a[: hi_i - lo_i]
    return out

